# revision 27
# baseline (speedup 1.0000x reference)
"""AnswerDecoder (LSTM decoder w/ visual attention) on 8 TRN2 NeuronCores.

Strategy: pure data-parallel over batch (8 batches/core), zero collectives.
Host does layout prep only (transposes/concat/embedding gather = data movement);
all FLOPs run on device.

Device algorithm per core (B_l = 8 local batches), bf16 matmul operands
(fp8e4m3 + DoubleRow for the recurrent gate matmuls when USE_FP8, with a
x64 weight scale folded back out through the gate activations), f32 PSUM
accumulation, f32 cell state:
  pre:   h0 = q @ Wh.T ; c0 = q @ Wc.T ; o0 = g @ Wg2o.T + b
         gy[(t,b)] = y_emb @ Wy.T + (b_ih+b_hh)  -> DRAM scratch (bf16)
         apT[h, (b,r)] = L @ W_attn.T            (K=H layout for e-matmul)
         pbd[(b,r), j] = L @ W_u[:, :F].T + b_u  (b_u folded: softmax sums to 1)
         full W_vocab resident in SBUF (bf16), b_vocab added on DVE
         (weights land in batched DMAs from host-prechunked [128,k,n] layouts;
          the ~600ns-per-descriptor issue path was the old pre bottleneck)
  iter t = 0..31:
         cell on gates(t) PSUM (all-Tanh trick, h'=2h / c'=2c) -> h bf16
         hT via PE transpose; E = hT.T@apT + mask (mask folded into the
         accumulation group); softmax without max-subtraction (|e| small)
         gates(t+1) = gy + h-part + o-part via PSUM accumulation, chunk
         order [g,i,f,og] so cell ACTs chase the matmuls
         vo = h@Wuh.T + abd.T@pbd ; o = tanh(vo) -> ot slot t+1 (bf16)
         vocab groups for steps 0..15 interleaved into iters 16..27 (keeps
         the PE warm through the attention/cell latency windows)
  post:  logits[(t,b), v] for steps 16..31 (group B) + b_vocab via DVE
"""

import numpy as np

B, T, R = 64, 32, 49
LOCAL, QVEC, EMB, HID, VOCAB = 1024, 512, 256, 512, 10000
START_IDX = 1
NCORES = 8
BL = B // NCORES        # 8 batches per core
ROWS = T * BL           # 256 output rows per core, t-major (row = t*BL + b)
NMM = [(0, 512), (512, 512), (1024, 226)]   # vocab col chunks per 1250
VCHUNK = 1250

USE_FP8 = False
GS = 64.0 if USE_FP8 else 1.0   # gate-matmul weight scale (fp8 resolution)


def _perm_gates():
    # reference gate order [i, f, g, o] -> device order [i, f, o, g]
    p = np.concatenate([np.arange(0, 512), np.arange(512, 1024),
                        np.arange(1536, 2048), np.arange(1024, 1536)])
    return p


def _chunk(a, p=128):
    """[K, N] -> [p, K//p, N] host permute so one DMA fills a [p,k,N] tile."""
    K, N = a.shape
    return np.ascontiguousarray(a.reshape(K // p, p, N).transpose(1, 0, 2))


def prep_inputs(image_local_features, image_global_features, question_vectors,
                answers, emb, W_g2o, b_g2o, W_h, W_c, W_ih, W_hh, b_ih, b_hh,
                W_attn, W_u, b_u, W_vocab, b_vocab):
    """Host-side data layout prep. Returns list of per-core input dicts."""
    import ml_dtypes

    f32 = np.float32
    bf16 = ml_dtypes.bfloat16
    fp8 = ml_dtypes.float8_e4m3
    L = np.ascontiguousarray(image_local_features, dtype=f32)       # [B,R,F]
    g = np.ascontiguousarray(image_global_features, dtype=f32)      # [B,2F]
    q = np.ascontiguousarray(question_vectors, dtype=f32)           # [B,Q]
    ans = np.asarray(answers).astype(np.int64)                      # [B,T]
    emb = np.asarray(emb, dtype=f32)

    perm = _perm_gates()
    # shared (batch-independent) weights
    Wy = W_ih[perm, :EMB].astype(f32)                               # [2048,256]
    bias_g = (b_ih + b_hh)[perm].astype(f32)                        # [2048]
    wy_aug = GS * np.concatenate([Wy.T, bias_g[None, :]], 0)        # [257,2048]
    wr = np.concatenate([W_ih[perm, EMB:EMB + HID], W_hh[perm]], 1).T  # [1024,2048]
    wr = GS * wr
    wr[HID:, :] *= 0.5        # h-part rows compensate h' = 2h
    wat = np.ascontiguousarray(0.5 * W_attn.astype(f32).T)          # [1024,512]
    wua = np.ascontiguousarray(W_u[:, :LOCAL].astype(f32).T)        # [1024,512]
    wuh = np.ascontiguousarray(0.5 * W_u[:, LOCAL:].astype(f32).T)  # [512,512]
    # h state is kept as h' = 2h and c as c' = 2c (sigmoid-via-tanh trick:
    # sigmoid(x) = (1+tanh(x/2))/2, so every ACT op is Tanh/Exp and the ACT
    # engine never swaps its function table). Weights touching h are halved;
    # wh/wc doubled so h0' = 2*h0, c0' = 2*c0.
    whc = np.concatenate([2.0 * W_h.astype(f32).T,
                          2.0 * W_c.astype(f32).T], 1)              # [512,1024]
    wg = W_g2o.astype(f32).T                                        # [2048,512]
    wv = np.ascontiguousarray(W_vocab.astype(f32).T)                # [512,10000]
    bvb = np.broadcast_to(np.asarray(b_vocab, f32), (128, VOCAB))   # [128,10000]
    bub = np.broadcast_to(np.asarray(b_u, f32), (128, HID))         # [128,512]

    mask = np.full((BL, BL * R), -1e9, f32)
    for b in range(BL):
        mask[b, b * R:(b + 1) * R] = 0.0
    ident = np.eye(128, dtype=f32)

    # teacher-forced input embeddings: y_seq[t] = emb[ans[:, t-1]], y_seq[0]=emb[1]
    idx = np.concatenate([np.full((B, 1), START_IDX, np.int64), ans[:, :-1]], 1)
    y_emb = emb[idx]                                                # [B,T,EMB]

    bb = lambda a: np.ascontiguousarray(np.asarray(a, f32).astype(bf16))
    # wr8 [128, d(4), plane(2), 2048]: DoubleRow pairs of K-chunks
    # (d0,d1 = o-part chunks (0,1),(2,3); d2,d3 = h-part chunks (4,5),(6,7))
    wr8 = wr.reshape(4, 2, 128, 2048).transpose(2, 0, 1, 3)
    in_maps = []
    for c in range(NCORES):
        s = slice(c * BL, (c + 1) * BL)
        # y_aug_T [257, 256]: rows t-major (t*BL+b), transposed, ones row
        ye = y_emb[s].transpose(1, 0, 2).reshape(ROWS, EMB)         # [(t,b), EMB]
        yT = ye.T                                                   # [256,256]
        lT = L[s].reshape(BL * R, LOCAL).T                          # [1024,392]
        qT = q[s].T                                                 # [512,8]
        gT = g[s].T                                                 # [2048,8]
        im = {
            "yT": bb(_chunk(yT)),
            "yTb": bb(np.ones((1, ROWS), f32)),
            "wy": bb(_chunk(wy_aug[:256])),
            "wyb": bb(wy_aug[256:257]),
            "lT": bb(_chunk(lT)),
            "wat": bb(_chunk(wat)),
            "wua": bb(_chunk(wua)),
            "wuh": bb(_chunk(wuh)),
            "qT": bb(_chunk(qT)),
            "whc": bb(_chunk(whc)),
            "gT": bb(_chunk(gT)),
            "gTb": bb(np.ones((1, BL), f32)),
            "wg": bb(_chunk(wg)),
            "wgb": bb(np.asarray(b_g2o, f32)[None, :]),
            "wv": bb(_chunk(wv)),
            "bvb": bb(bvb),
            "bub": bb(bub),
            "mask": bb(mask),
            "ident": bb(ident),
        }
        if USE_FP8:
            im["wr8"] = np.ascontiguousarray(wr8.astype(fp8))
        else:
            im["wr"] = bb(_chunk(wr))
        in_maps.append(im)
    return in_maps


def build_nc():
    """Build the per-core Bass graph (identical on all 8 cores)."""
    from concourse import bacc, tile
    import concourse.mybir as mybir

    f32 = mybir.dt.float32
    bf16 = mybir.dt.bfloat16
    fp8 = mybir.dt.float8e4
    AF = mybir.ActivationFunctionType
    ALU = mybir.AluOpType
    DR = mybir.MatmulPerfMode.DoubleRow

    nc = bacc.Bacc("TRN2", target_bir_lowering=False, debug=False,
                   num_devices=NCORES)

    def dparam(name, shape, dt=bf16):
        return nc.dram_tensor(name, list(shape), dt, kind="ExternalInput").ap()

    yT_d = dparam("yT", [128, 2, 256])
    yTb_d = dparam("yTb", [1, 256])
    wy_d = dparam("wy", [128, 2, 2048])
    wyb_d = dparam("wyb", [1, 2048])
    if USE_FP8:
        wr8_d = dparam("wr8", [128, 4, 2, 2048], fp8)
    else:
        wr_d = dparam("wr", [128, 8, 2048])
    lT_d = dparam("lT", [128, 8, BL * R])
    wat_d = dparam("wat", [128, 8, 512])
    wua_d = dparam("wua", [128, 8, 512])
    wuh_d = dparam("wuh", [128, 4, 512])
    qT_d = dparam("qT", [128, 4, BL])
    whc_d = dparam("whc", [128, 4, 1024])
    gT_d = dparam("gT", [128, 16, BL])
    gTb_d = dparam("gTb", [1, BL])
    wg_d = dparam("wg", [128, 16, 512])
    wgb_d = dparam("wgb", [1, 512])
    wv_d = dparam("wv", [128, 4, VOCAB])
    bvb_d = dparam("bvb", [128, VOCAB])
    bub_d = dparam("bub", [128, HID])
    mask_d = dparam("mask", [BL, BL * R])
    id_d = dparam("ident", [128, 128])
    out_d = nc.dram_tensor("out", [ROWS, VOCAB], bf16,
                       kind="ExternalOutput").ap()

    BR = BL * R  # 392
    # (b,r) partition-tile sizes: 128,128,128,8
    brsz = [128, 128, 128, BR - 384]
    CHORD = [3, 0, 1, 2]          # gate chunk order [g, i, f, og]
    ADD, MULT = ALU.add, ALU.mult

    with tile.TileContext(nc) as tc:
        import contextlib
        stack = contextlib.ExitStack()
        with stack:
            pers = stack.enter_context(tc.tile_pool(name="pers", bufs=1))
            stb1 = stack.enter_context(tc.tile_pool(name="stb1", bufs=1))
            stb2 = stack.enter_context(tc.tile_pool(name="stb2", bufs=2))
            lpool = stack.enter_context(tc.tile_pool(name="lg", bufs=6))
            lypool = stack.enter_context(tc.tile_pool(name="lgy", bufs=2))
            gyp = stack.enter_context(tc.tile_pool(name="gyp", bufs=2))
            gyd = stack.enter_context(tc.tile_pool(name="gyd", bufs=1, space="DRAM"))
            pmm = stack.enter_context(tc.tile_pool(name="pmm", bufs=3, space="PSUM"))
            ptp = stack.enter_context(tc.tile_pool(name="ptp", bufs=1, space="PSUM"))
            pgstack = contextlib.ExitStack()
            pg = pgstack.enter_context(
                tc.tile_pool(name="pg", bufs=1, space="PSUM"))

            # ---- persistent SBUF ----
            if USE_FP8:
                wr8_sb = pers.tile([128, 4, 2, 2048], fp8)
            else:
                wr_sb = pers.tile([128, 8, 2048], bf16)
            gy_dram = gyd.tile([ROWS, 2048], bf16)
            apT_sb = pers.tile([128, 4, BR], bf16)
            pbd_sb = pers.tile([128, 4, 512], bf16)
            ot_sb = pers.tile([128, 4, T + 1, BL], bf16)
            ot8_sb = None
            if USE_FP8:
                ot8_sb = pers.tile([128, 4, T + 1, 16], fp8, name="ot8_sb")
            wuh_sb = pers.tile([128, 4, 512], bf16)
            wv_sb = pers.tile([128, 4, VOCAB], bf16)
            bvb_sb = pers.tile([128, VOCAB], bf16)
            mask_sb = pers.tile([BL, BR], bf16)
            id_sb = pers.tile([128, 128], bf16)
            c_sb = pers.tile([BL, 512], f32)

            dma = nc.sync.dma_start
            dmaa = nc.scalar.dma_start
            dmag = nc.gpsimd.dma_start
            id8 = id_sb[0:8, 0:8]
            mm = nc.tensor.matmul

            # ---- precompute: h0/c0 (+ the big weight loads queued behind;
            #      each load is one batched DMA from a host-prechunked layout,
            #      split only to spread across DMA engines) ----
            with tc.tile_pool(name="preA1", bufs=1) as preA1:
                q_sb = preA1.tile([128, 4, BL], bf16)
                whc_sb = preA1.tile([128, 4, 1024], bf16)
                h0_sb = preA1.tile([BL, 512], bf16)
                dma(out=mask_sb[:, :], in_=mask_d[:, :])
                dma(out=id_sb[:, :], in_=id_d[:, :])
                dma(out=q_sb[:, :, :], in_=qT_d[:, :, :])
                if USE_FP8:
                    for dk in range(4):
                        dmag(out=wr8_sb[:, dk, :, :], in_=wr8_d[:, dk, :, :])
                else:
                    for dk in range(8):
                        dmag(out=wr_sb[:, dk:dk + 1, :],
                             in_=wr_d[:, dk:dk + 1, :])
                for k in range(4):
                    dma(out=whc_sb[:, k, 0:512], in_=whc_d[:, k, 0:512])
                    dma(out=whc_sb[:, k, 512:1024], in_=whc_d[:, k, 512:1024])
                ps = pmm.tile([BL, 512], f32, tag="mm")
                for k in range(4):
                    mm(ps[:, :], q_sb[:, k, :], whc_sb[:, k, 0:512],
                       start=(k == 0), stop=(k == 3))
                nc.scalar.copy(h0_sb[:, :], ps[:, :])
                ps2 = pmm.tile([BL, 512], f32, tag="mm")
                for k in range(4):
                    mm(ps2[:, :], q_sb[:, k, :], whc_sb[:, k, 512:1024],
                       start=(k == 0), stop=(k == 3))
                nc.vector.tensor_copy(c_sb[:, :], ps2[:, :])
                tp = ptp.tile([128, 4, BL], bf16, tag="tp")
                for j in range(4):
                    nc.tensor.transpose(tp[:, j, :],
                                        h0_sb[:, j * 128:(j + 1) * 128], id8)
                hT = stb2.tile([128, 4, BL], bf16, tag="hT")
                nc.vector.tensor_copy(hT[:, :, :], tp[:, :, :])
                if USE_FP8:
                    hT8 = stb2.tile([128, 4, 16], fp8, tag="hT8")
                    nc.vector.tensor_copy(hT8[:, :, 0:BL], tp[:, :, :])

            # ---- precompute: o0 ----
            with tc.tile_pool(name="preA2", bufs=1) as preA2:
                g_sb = preA2.tile([128, 16, BL], bf16)
                gb_sb = preA2.tile([1, BL], bf16)
                wgb_sb = preA2.tile([1, 512], bf16)
                dma(out=g_sb[:, :, :], in_=gT_d[:, :, :])
                dma(out=gb_sb[:, :], in_=gTb_d[:, :])
                # wg parks in wv_sb's space (wv loads are dep-ordered behind
                # the o0 reads and have ~200us of slack)
                for k in range(8):
                    dma(out=wv_sb[:, 0, k * 1024:(k + 1) * 1024],
                        in_=wg_d[:, 2 * k:2 * k + 2, :])
                dma(out=wgb_sb[:, :], in_=wgb_d[:, :])
                ps = pmm.tile([BL, 512], f32, tag="mm")
                mm(ps[:, :], gb_sb[:, :], wgb_sb[:, :], start=True, stop=False)
                for k in range(16):
                    mm(ps[:, :], g_sb[:, k, :],
                       wv_sb[:, 0, k * 512:(k + 1) * 512],
                       start=False, stop=(k == 15))
                o0_sb = stb2.tile([BL, 512], bf16, tag="o")
                nc.scalar.copy(o0_sb[:, :], ps[:, :])
                tp = ptp.tile([128, 4, BL], bf16, tag="tp")
                for j in range(4):
                    nc.tensor.transpose(tp[:, j, :], o0_sb[:, j * 128:(j + 1) * 128],
                                        id8)
                nc.vector.tensor_copy(ot_sb[:, :, 0, :], tp[:, :, :])
                if USE_FP8:
                    nc.vector.tensor_copy(ot8_sb[:, :, 0, 0:BL], tp[:, :, :])

            # ---- precompute: gates_y (steps 0..15 only; 16..31 runs in
            #      loop iters 0..3) -> DRAM scratch ----
            # (per-step slices are re-loaded at partition base 0, a matmul
            #  operand requirement)
            y_sb = pers.tile([128, 2, 256], bf16)
            yb_sb = pers.tile([1, 256], bf16)
            wy_sb = pers.tile([128, 2, 2048], bf16)
            wyb_sb = pers.tile([1, 2048], bf16)
            dma(out=y_sb[:, :, :], in_=yT_d[:, :, :])
            dma(out=yb_sb[:, :], in_=yTb_d[:, :])
            for k in range(2):
                dma(out=wy_sb[:, k, 0:1024], in_=wy_d[:, k, 0:1024])
                dma(out=wy_sb[:, k, 1024:2048], in_=wy_d[:, k, 1024:2048])
            dma(out=wyb_sb[:, :], in_=wyb_d[:, :])
            for j in range(4):
                ps = pmm.tile([128, 512], f32, tag="mm")
                js = slice(j * 512, (j + 1) * 512)
                mm(ps[:, :], yb_sb[:, 0:128], wyb_sb[:, js],
                   start=True, stop=False)
                for k in range(2):
                    mm(ps[:, :], y_sb[:, k, 0:128], wy_sb[:, k, js],
                       start=False, stop=(k == 1))
                lg = lypool.tile([128, 512], bf16, tag="lgy")
                nc.vector.tensor_copy(lg[:, :], ps[:, :])
                dma(out=gy_dram[0:128, js], in_=lg[:, :])

            # gate-matmul emitters (shared by gates(0) and the loop)
            def gy_h_part(g_t, gyt, hT, hT8, j):
                js = slice(j * 512, (j + 1) * 512)
                mm(g_t[j][0:8, :], id8, gyt[0:8, js],
                   start=True, stop=False, skip_group_check=True)
                if USE_FP8:
                    for dk in range(2):
                        mm(g_t[j][:, :], hT8[:, 2 * dk:2 * dk + 2, :],
                           wr8_sb[:, 2 + dk, :, js], perf_mode=DR,
                           start=False, stop=False, skip_group_check=True)
                else:
                    for k in range(4):
                        mm(g_t[j][0:8, :], hT[:, k, :], wr_sb[:, 4 + k, js],
                           start=False, stop=False, skip_group_check=True)

            def o_part(g_t, slot, j):
                js = slice(j * 512, (j + 1) * 512)
                if USE_FP8:
                    for dk in range(2):
                        mm(g_t[j][:, :], ot8_sb[:, 2 * dk:2 * dk + 2, slot, :],
                           wr8_sb[:, dk, :, js], perf_mode=DR,
                           start=False, stop=(dk == 1), skip_group_check=True)
                else:
                    for k in range(4):
                        mm(g_t[j][0:8, :], ot_sb[:, k, slot, :], wr_sb[:, k, js],
                           start=False, stop=(k == 3), skip_group_check=True)

            # ---- precompute: apT + pbd ----
            with tc.tile_pool(name="preC", bufs=1) as preC:
                # lT/wat/wua park in wv_sb rows 1..3 (same slack argument)
                lT_a = wv_sb[:, 1, 0:8 * BR]
                wat_a = wv_sb[:, 2, 0:4096]
                wua_a = wv_sb[:, 3, 0:4096]
                for k in range(4):
                    dma(out=wv_sb[:, 1, 2 * k * BR:2 * (k + 1) * BR],
                        in_=lT_d[:, 2 * k:2 * k + 2, :])
                for k in range(4):
                    dma(out=wv_sb[:, 2, k * 1024:(k + 1) * 1024],
                        in_=wat_d[:, 2 * k:2 * k + 2, :])
                bub_sb = preC.tile([128, 512], bf16)
                dma(out=bub_sb[:, :], in_=bub_d[:, :])
                for k in range(4):
                    dma(out=wv_sb[:, 3, k * 1024:(k + 1) * 1024],
                        in_=wua_d[:, 2 * k:2 * k + 2, :])
                dma(out=wuh_sb[:, 0:2, :], in_=wuh_d[:, 0:2, :])
                dma(out=wuh_sb[:, 2:4, :], in_=wuh_d[:, 2:4, :])

                for hk in range(4):
                    ps = pmm.tile([128, BR], f32, tag="mm")
                    for k in range(8):
                        mm(ps[:, :],
                           wv_sb[:, 2, k * 512 + hk * 128:k * 512 + (hk + 1) * 128],
                           wv_sb[:, 1, k * BR:(k + 1) * BR],
                           start=(k == 0), stop=(k == 7))
                    nc.vector.tensor_copy(apT_sb[:, hk, :], ps[:, :])
                for mt in range(4):
                    sz = brsz[mt]
                    ps = pmm.tile([128, 512], f32, tag="mm")
                    for k in range(8):
                        mm(ps[0:sz, :],
                           wv_sb[:, 1, k * BR + mt * 128:k * BR + mt * 128 + sz],
                           wv_sb[:, 3, k * 512:(k + 1) * 512],
                           start=(k == 0), stop=(k == 7))
                    # b_u folded here: att rows sum to 1, so adding b_u to
                    # every pbd row injects the bias through the att matmul
                    nc.vector.tensor_add(pbd_sb[0:sz, mt, :], ps[0:sz, :],
                                         bub_sb[0:sz, :])

            # ---- gates(0) ----
            gyt = gyp.tile([BL, 2048], bf16, tag="gyt")
            dma(out=gyt[:, :], in_=gy_dram[0:BL, :])
            g_ps = [pg.tile([16, 512], f32, name=f"gps{j}", tag=f"g{j}")
                    for j in range(4)]
            for j in range(4):
                gy_h_part(g_ps, gyt, hT, hT8 if USE_FP8 else None, j)
                o_part(g_ps, 0, j)

            # vocab emission helper: one (vc, m, col-chunk) group = 4 matmuls
            # + bias add + store
            def vocab_group(pool, tag, vc, m, nb, nsz):
                vb = vc * VCHUNK
                ps = pool.tile([128, 512], f32, tag=tag, name="vps")
                for k in range(4):
                    mm(ps[:, 0:nsz], ot_sb[:, k, 1 + m * 16:1 + m * 16 + 16, :],
                       wv_sb[:, k, vb + nb:vb + nb + nsz],
                       start=(k == 0), stop=(k == 3))
                lg = lpool.tile([128, 512], bf16, tag="lg")
                nc.vector.tensor_add(lg[:, 0:nsz], ps[:, 0:nsz],
                                     bvb_sb[:, vb + nb:vb + nb + nsz])
                dma(out=out_d[m * 128:(m + 1) * 128, vb + nb:vb + nb + nsz],
                    in_=lg[:, 0:nsz])

            groups_a = [(vc, 0, nb, nsz) for vc in range(VOCAB // VCHUNK)
                        for (nb, nsz) in NMM]
            groups_b = [(vc, 1, nb, nsz) for vc in range(VOCAB // VCHUNK)
                        for (nb, nsz) in NMM]

            # ---- recurrence ----
            stt = nc.vector.scalar_tensor_tensor
            for t in range(T):
                # gy(t+1) fetch first (vector DGE queue): ~2us DMA latency,
                # needed mid-iter
                if t < T - 1:
                    gyt = gyp.tile([BL, 2048], bf16, tag="gyt")
                    dma(out=gyt[:, :], in_=gy_dram[(t + 1) * BL:(t + 2) * BL, :])
                # vocab weights/bias stream during the early loop (issued
                # here so they can't steal DMA rings from the pre loads)
                if 1 <= t <= 8:
                    wk, wh2 = divmod(t - 1, 2)
                    dma(out=wv_sb[:, wk, wh2 * 5000:(wh2 + 1) * 5000],
                        in_=wv_d[:, wk, wh2 * 5000:(wh2 + 1) * 5000])
                if 5 <= t <= 8:
                    bh = t - 5
                    dmag(out=bvb_sb[:, bh * 2500:(bh + 1) * 2500],
                         in_=bvb_d[:, bh * 2500:(bh + 1) * 2500])

                # LSTM cell on gates(t), all-Tanh form. State: c_sb=2c, h=2h.
                # Gate matmuls carry a GS scale (fp8 resolution); the ACT
                # scale folds it back out.
                tg = stb1.tile([BL, 512], f32, tag="tg")
                nc.scalar.activation(tg[:, :], g_ps[3][0:8, :], AF.Tanh,
                                     scale=1.0 / GS)
                ti = stb1.tile([BL, 512], f32, tag="ti")
                nc.scalar.activation(ti[:, :], g_ps[0][0:8, :], AF.Tanh,
                                     scale=0.5 / GS)
                tf_ = stb1.tile([BL, 512], f32, tag="tf")
                nc.scalar.activation(tf_[:, :], g_ps[1][0:8, :], AF.Tanh,
                                     scale=0.5 / GS)
                tog = stb1.tile([BL, 512], f32, tag="tog")
                nc.scalar.activation(tog[:, :], g_ps[2][0:8, :], AF.Tanh,
                                     scale=0.5 / GS)
                t1 = stb1.tile([BL, 512], f32, tag="t1")
                stt(t1[:, :], ti[:, :], 1.0, tg[:, :], op0=ADD, op1=MULT)
                m2 = stb1.tile([BL, 512], f32, tag="m2")
                stt(m2[:, :], tf_[:, :], 1.0, c_sb[:, :], op0=ADD, op1=MULT)
                stt(c_sb[:, :], m2[:, :], 0.5, t1[:, :], op0=MULT, op1=ADD)
                tc2 = stb1.tile([BL, 512], f32, tag="tc2")
                nc.scalar.activation(tc2[:, :], c_sb[:, :], AF.Tanh, scale=0.5)
                h = stb2.tile([BL, 512], bf16, tag="h")
                stt(h[:, :], tog[:, :], 1.0, tc2[:, :], op0=ADD, op1=MULT)

                tp = ptp.tile([128, 4, BL], bf16, tag="tp")
                for j in range(4):
                    nc.tensor.transpose(tp[:, j, :], h[:, j * 128:(j + 1) * 128],
                                        id8)
                hT = stb2.tile([128, 4, BL], bf16, tag="hT")
                nc.vector.tensor_copy(hT[:, :, :], tp[:, :, :])
                if USE_FP8:
                    hT8 = stb2.tile([128, 4, 16], fp8, tag="hT8")
                    nc.vector.tensor_copy(hT8[:, :, 0:BL], tp[:, :, :])

                # e matmul -> E_full [8, 392]; mask folded into the group
                E = pmm.tile([BL, BR], f32, tag="mm")
                mm(E[:, :], id8, mask_sb[:, :], start=True, stop=False)
                for k in range(4):
                    mm(E[:, :], hT[:, k, :], apT_sb[:, k, :],
                       start=False, stop=(k == 3))

                # vo h-part early (hT-only dependency)
                vo = pmm.tile([BL, 512], f32, tag="mm")
                for k in range(4):
                    mm(vo[:, :], hT[:, k, :], wuh_sb[:, k, :],
                       start=(k == 0), stop=False, skip_group_check=True)

                # softmax without max-subtraction (|e| <~ 35, f32-safe);
                # expv bf16 for 2x DVE and bf16 transposes downstream
                expv = stb2.tile([BL, BR], bf16, tag="expv")
                ssum = stb2.tile([BL, 1], f32, tag="ssum")
                nc.scalar.activation(expv[:, :], E[:, :], AF.Exp,
                                     accum_out=ssum[:, :])
                rs = stb2.tile([BL, 1], f32, tag="rs")
                nc.vector.reciprocal(rs[:, :], ssum[:, :])
                att = stb2.tile([BL, BR], bf16, tag="att")
                nc.vector.tensor_scalar_mul(att[:, :], expv[:, :], rs[:, :])

                # gates(t+1): gy + h-part for chunks g, i (PE fills the
                # softmax latency window)
                if t < T - 1:
                    g_next = [pg.tile([16, 512], f32, name=f"gnx{j}", tag=f"g{j}")
                              for j in range(4)]
                    gy_h_part(g_next, gyt, hT, hT8 if USE_FP8 else None, CHORD[0])
                    gy_h_part(g_next, gyt, hT, hT8 if USE_FP8 else None, CHORD[1])

                # transpose att -> block-diagonal [(b,r), b]
                tpa = ptp.tile([128, 4, BL], bf16, tag="tp")
                for j in range(4):
                    sz = brsz[j]
                    nc.tensor.transpose(tpa[0:sz, j, :],
                                        att[:, j * 128:j * 128 + sz], id8)
                abd = stb2.tile([128, 4, BL], bf16, tag="abd")
                nc.vector.tensor_copy(abd[:, 0:3, :], tpa[:, 0:3, :])
                nc.vector.tensor_copy(abd[0:8, 3, :], tpa[0:8, 3, :])

                # vo += att-weighted P (b_u already folded into pbd)
                for j in range(4):
                    sz = brsz[j]
                    mm(vo[:, :], abd[0:sz, j, :], pbd_sb[0:sz, j, :],
                       start=False, stop=(j == 3), skip_group_check=True)

                if t < T - 1:
                    gy_h_part(g_next, gyt, hT, hT8 if USE_FP8 else None, CHORD[2])

                o_sb = stb2.tile([BL, 512], bf16, tag="o")
                nc.scalar.activation(o_sb[:, :], vo[:, :], AF.Tanh)
                tpo = ptp.tile([128, 4, BL], bf16, tag="tp")
                for j in range(4):
                    nc.tensor.transpose(tpo[:, j, :], o_sb[:, j * 128:(j + 1) * 128],
                                        id8)
                nc.vector.tensor_copy(ot_sb[:, :, t + 1, :], tpo[:, :, :])
                if USE_FP8:
                    nc.vector.tensor_copy(ot8_sb[:, :, t + 1, 0:BL], tpo[:, :, :])

                if t < T - 1:
                    gy_h_part(g_next, gyt, hT, hT8 if USE_FP8 else None, CHORD[3])
                    # gates(t+1): o-part, chunk g first so cell ACTs of the
                    # next iteration chase the accumulation
                    for j in CHORD:
                        o_part(g_next, t + 1, j)
                    g_ps = g_next

                # vocab group A (steps 0..15) interleaved once slot 16 exists;
                # fills the cell-tail PE idle window and keeps HAM warm
                if t < 4:
                    # gy for steps 16..31 (j-group t), moved out of pre into
                    # the early-iteration PE idle windows
                    js = slice(t * 512, (t + 1) * 512)
                    psy = pmm.tile([128, 512], f32, tag="mm")
                    mm(psy[:, :], yb_sb[:, 128:256], wyb_sb[:, js],
                       start=True, stop=False)
                    for k in range(2):
                        mm(psy[:, :], y_sb[:, k, 128:256], wy_sb[:, k, js],
                           start=False, stop=(k == 1))
                    lgy = lypool.tile([128, 512], bf16, tag="lgy")
                    nc.vector.tensor_copy(lgy[:, :], psy[:, :])
                    dma(out=gy_dram[128:256, js], in_=lgy[:, :])
                if 16 <= t < 28:
                    i = 2 * (t - 16)
                    vocab_group(pmm, "mm", *groups_a[i])
                    vocab_group(pmm, "mm", *groups_a[i + 1])

            # ---- vocab projection: group B tail ----
            # gate banks are free now; deeper rotation decouples the matmul
            # stream from the bias-add/store pipeline
            pgstack.close()
            with tc.tile_pool(name="pvt", bufs=4, space="PSUM") as pvt:
                for grp in groups_b:
                    vocab_group(pvt, "vmm", *grp)

    nc.compile()
    return nc


_STATE = {}


def kernel(**inputs):
    from concourse.bass_utils import run_bass_kernel_spmd

    in_maps = prep_inputs(**inputs)
    if "nc" not in _STATE:
        _STATE["nc"] = build_nc()
    nc = _STATE["nc"]
    res = run_bass_kernel_spmd(nc, in_maps, core_ids=list(range(NCORES)))
    full = np.empty((B, T, VOCAB), np.float32)
    for c in range(NCORES):
        full[c * BL:(c + 1) * BL] = (
            res.results[c]["out"].astype(np.float32)
            .reshape(T, BL, VOCAB).transpose(1, 0, 2))
    return full
